# revision 24
# baseline (speedup 1.0000x reference)
"""Trainium2 Bass kernel for nn_MLP_Interpolate.

Reference computation (out_size=512, H=W=128 -> exact 4x nearest upsample):
  out[b, :, 4i+r, 4l+s] = relu(x[b,:,i,l] @ W1[:64] + c[r,s]) @ W2 + b2
  c[r,s] = rel_y(r)*W1[64] + rel_x(s)*W1[65] + b1,  rel(t) = (2t-3)/4

Strategy (8 cores, shard = (batch, H-half)), all-bf16 datapath:
  - stage 1 (features F = W1c^T x): one bf16 matmul per 1024 pixels with a
    block-diagonal [128,128] stationary so two 64-channel pixel groups
    (A = top half rows, B = bottom half) share one pass.
  - ACT copies F (PSUM fp32) -> bf16 SBUF; DVE then computes all 16
    bias+relu variants as bf16 tensor_scalar ops (4x perf mode).
  - stage 2 (pred = h @ W2): [128,6] block-diag W2 stationary placed on 4
    independent PE column strips via tile_position=(0,32r) so 4 matmuls
    stream concurrently. Moving operand reads h with the (l,s) interleave
    so PSUM rows are whole output rows.
  - ACT copies pred (PSUM) -> bf16 staging; two DMA queues (sync/gpsimd)
    write 48KB batched descriptors straight to DRAM. Host upcasts + b2.
"""

import numpy as np
import ml_dtypes

import concourse.bass as bass
import concourse.bacc as bacc
import concourse.mybir as mybir
import concourse.tile as tile
from concourse.bass_utils import run_bass_kernel_spmd

# Problem constants (hardcoded per contract)
B, C, H, W = 4, 64, 128, 128
OUT = 512
NF = 64
N_CORES = 8
ROWS_PER_CORE = H // 2          # 64 input rows per core
NBLK = 2                        # blocks per core; block = 16 A-rows + 16 B-rows
BI = 16                         # input rows per group per block
REL = np.array([-0.75, -0.25, 0.25, 0.75], dtype=np.float32)

_CACHE = {}


def _build_program():
    if "nc" in _CACHE:
        return _CACHE["nc"]

    fp32 = mybir.dt.float32
    bf16 = mybir.dt.bfloat16
    nc = bacc.Bacc("TRN2", target_bir_lowering=False, debug=False,
                   num_devices=N_CORES)

    x_d = nc.dram_tensor("x", [C, ROWS_PER_CORE, W], bf16, kind="ExternalInput")
    w1_d = nc.dram_tensor("w1diag", [128, 128], bf16, kind="ExternalInput")
    w2_d = nc.dram_tensor("w2diag", [128, 6], bf16, kind="ExternalInput")
    crs_d = nc.dram_tensor("crsT", [128, 16], fp32, kind="ExternalInput")
    out_d = nc.dram_tensor("out", [3, 4 * ROWS_PER_CORE, OUT], bf16,
                           kind="ExternalOutput")

    with tile.TileContext(nc) as tc:
        with (
            tc.tile_pool(name="consts", bufs=1) as consts,
            tc.tile_pool(name="xin", bufs=2) as xin,
            tc.tile_pool(name="fbf", bufs=2) as fbfp,
            tc.tile_pool(name="hbuf", bufs=2) as hbuf,
            tc.tile_pool(name="stage", bufs=2) as stage,
            tc.tile_pool(name="fpsum", bufs=2, space=bass.MemorySpace.PSUM) as fpsum,
            tc.tile_pool(name="ppsum", bufs=2, space=bass.MemorySpace.PSUM) as ppsum,
        ):
            w1_sb = consts.tile([128, 128], bf16)
            w2_sb = consts.tile([128, 6], bf16)
            crs_sb = consts.tile([128, 16], fp32)

            x_tiles, f_tiles, fbf_tiles = [], [], []

            def load_x(b, eng_a=None, eng_b=None):
                xt = xin.tile([128, BI, W], bf16, tag="xt")
                (eng_a or nc.sync).dma_start(
                    xt[0:64, :, :], x_d[:, BI * b:BI * (b + 1), :])
                (eng_b or nc.gpsimd).dma_start(
                    xt[64:128, :, :],
                    x_d[:, 32 + BI * b:32 + BI * (b + 1), :])
                x_tiles.append(xt)

            # x(0) entirely on the sync HWDGE queue (lower completion
            # latency than SWDGE -> stage-1 starts sooner); w1 next on sync;
            # w2/crs + x(1) on gpsimd
            load_x(0, eng_a=nc.sync, eng_b=nc.sync)
            nc.sync.dma_start(w1_sb[:], w1_d[:])
            nc.gpsimd.dma_start(w2_sb[:], w2_d[:])
            nc.gpsimd.dma_start(crs_sb[:], crs_d[:])

            # PE clock-gate warmup: dummy matmuls on zeroed scratch while
            # the x DMA is in flight, so stage-1 runs at full 2.4 GHz
            scr = consts.tile([128, 512], bf16)
            nc.vector.memset(scr[:], 0.0)
            for _ in range(12):
                pw = fpsum.tile([128, 8, W], fp32, tag="ft")
                nc.tensor.matmul(pw[:, 0:4, :], scr[:, 0:128], scr[:, :],
                                 start=True, stop=True)

            def feat(b):
                # two 1024-col stage-1 matmuls -> F halves in PSUM
                xt = x_tiles[b]
                fs = []
                for half in range(2):
                    ft = fpsum.tile([128, 8, W], fp32, tag="ft")
                    for q in range(2):
                        sl = slice(4 * q, 4 * q + 4)
                        nc.tensor.matmul(ft[:, sl, :], w1_sb[:],
                                         xt[:, 8 * half:8 * half + 8, :][:, sl, :],
                                         start=True, stop=True)
                    fs.append(ft)
                f_tiles.append(fs)

            def fbf_copy(b):
                # ACT: PSUM fp32 -> SBUF bf16 (feeds DVE 4x relu)
                fb = fbfp.tile([128, 2 * 8 * W], bf16, tag="fb")
                for half in range(2):
                    nc.scalar.activation(
                        fb[:, 1024 * half:1024 * (half + 1)],
                        f_tiles[b][half][:, :, :],
                        mybir.ActivationFunctionType.Copy)
                fbf_tiles.append(fb)

            def body(b):
                fb = fbf_tiles[b]
                # DVE: 16 bias+relu variants, each [128, 2048] bf16 (4x mode)
                h = hbuf.tile([128, 16, 2048], bf16, tag="h")
                for v in range(16):
                    nc.vector.tensor_scalar(
                        h[:, v, :], fb[:, :],
                        crs_sb[:, v:v + 1], 0.0,
                        mybir.AluOpType.add, mybir.AluOpType.max)

                # stage 2, r-major: an r-quarter only needs relu variants
                # 4r..4r+3, so PE consumes h incrementally as DVE produces
                # it. Strip = i//4, pt slot k = i%4: partition-group G then
                # holds output rows 16G+4k+r -> contiguous DRAM rows.
                # st free layout: (k, r, col).
                st = stage.tile([102, 4, 4, OUT], bf16, tag="st")
                for r in range(4):
                    for kp in range(2):
                        pt = ppsum.tile([102, 2, OUT], fp32, tag="pt")
                        for kk in range(2):
                            for G in range(4):
                                i = 4 * G + 2 * kp + kk
                                rhs = h[:, 4 * r:4 * r + 4,
                                        128 * i:128 * (i + 1)]
                                nc.tensor.matmul(pt[32 * G:32 * G + 6, kk, :],
                                                 w2_sb[:], rhs,
                                                 start=True, stop=True,
                                                 tile_position=(0, 32 * G))
                        # un-interleave (s,l) -> (4l+s) via strided PSUM read
                        src = pt[:, :, :].rearrange("p j (s l) -> p j l s",
                                                    s=4)
                        dst = st[:, 2 * kp:2 * kp + 2, r, :]
                        if b == NBLK - 1 and r == 3 and kp == 0:
                            # DVE is done with relus by now; split the final
                            # two copies across both engines
                            nc.vector.tensor_copy(dst, src)
                        else:
                            nc.scalar.activation(
                                dst, src, mybir.ActivationFunctionType.Copy)

                # output DMAs: rows 128ab + 64b + 16G + (4k+r) are contiguous
                # 16-row runs; last block drains in k-halves
                ksplit = ((0, 2), (2, 4)) if b == NBLK - 1 else ((0, 4),)
                for k0, k1 in ksplit:
                    for idx, (G, ab) in enumerate(
                            (G, ab) for G in range(4) for ab in range(2)):
                        row0 = 128 * ab + 64 * b + 16 * G + 4 * k0
                        eng = nc.gpsimd if (G + ab) % 2 else nc.sync
                        eng.dma_start(
                            out_d[:, row0:row0 + 4 * (k1 - k0), :],
                            st[32 * G + 3 * ab:32 * G + 3 * ab + 3,
                               k0:k1, :, :])

            feat(0)
            fbf_copy(0)
            for b in range(NBLK):
                if b + 1 < NBLK:
                    load_x(b + 1, eng_a=nc.gpsimd, eng_b=nc.gpsimd)
                    feat(b + 1)
                    fbf_copy(b + 1)
                body(b)

    nc.compile()
    _CACHE["nc"] = nc
    return nc


def _prep_inputs(x, W1, b1, W2, b2):
    x = np.asarray(x, dtype=np.float32)
    W1 = np.asarray(W1, dtype=np.float32)
    b1 = np.asarray(b1, dtype=np.float32)
    W2 = np.asarray(W2, dtype=np.float32)

    w1c = W1[:NF]
    w1diag = np.zeros((128, 128), dtype=np.float32)
    w1diag[0:64, 0:64] = w1c
    w1diag[64:128, 64:128] = w1c

    w2diag = np.zeros((128, 6), dtype=np.float32)
    w2diag[0:64, 0:3] = W2
    w2diag[64:128, 3:6] = W2

    # c[r,s] = rel[r]*W1[64] + rel[s]*W1[65] + b1 -> [16, 64] -> [128, 16]
    crs = (REL[:, None, None] * W1[NF][None, None, :]
           + REL[None, :, None] * W1[NF + 1][None, None, :]
           + b1[None, None, :]).reshape(16, NF)
    crsT = np.ascontiguousarray(np.concatenate([crs.T, crs.T], axis=0))

    w1_bf = w1diag.astype(ml_dtypes.bfloat16)
    w2_bf = w2diag.astype(ml_dtypes.bfloat16)

    in_maps = []
    for c in range(N_CORES):
        b, half = c // 2, c % 2
        xs = np.ascontiguousarray(
            x[b, :, half * ROWS_PER_CORE:(half + 1) * ROWS_PER_CORE, :]
        ).astype(ml_dtypes.bfloat16)
        in_maps.append({"x": xs, "w1diag": w1_bf, "w2diag": w2_bf,
                        "crsT": crsT})
    return in_maps


def _gather(results, b2):
    full = np.empty((B, 3, OUT, OUT), dtype=np.float32)
    for c in range(N_CORES):
        b, half = c // 2, c % 2
        full[b, :, half * (OUT // 2):(half + 1) * (OUT // 2), :] = \
            np.asarray(results[c]["out"]).astype(np.float32)
    b2 = np.asarray(b2, dtype=np.float32)
    if np.any(b2):
        full += b2.reshape(1, 3, 1, 1)
    return full


def run(trace=False, **inputs):
    nc = _build_program()
    in_maps = _prep_inputs(inputs["x"], inputs["W1"], inputs["b1"],
                           inputs["W2"], inputs["b2"])
    res = run_bass_kernel_spmd(nc, in_maps, list(range(N_CORES)), trace=trace)
    return _gather(res.results, inputs["b2"]), res


def kernel(**inputs):
    out, _ = run(trace=False, **inputs)
    return out


# revision 25
# speedup vs baseline: 1.0087x; 1.0087x over previous
"""Trainium2 Bass kernel for nn_MLP_Interpolate.

Reference computation (out_size=512, H=W=128 -> exact 4x nearest upsample):
  out[b, :, 4i+r, 4l+s] = relu(x[b,:,i,l] @ W1[:64] + c[r,s]) @ W2 + b2
  c[r,s] = rel_y(r)*W1[64] + rel_x(s)*W1[65] + b1,  rel(t) = (2t-3)/4

Strategy (8 cores, shard = (batch, H-half)), all-bf16 datapath:
  - stage 1 (features F = W1c^T x): one bf16 matmul per 1024 pixels with a
    block-diagonal [128,128] stationary so two 64-channel pixel groups
    (A = top half rows, B = bottom half) share one pass.
  - ACT copies F (PSUM fp32) -> bf16 SBUF; DVE then computes all 16
    bias+relu variants as bf16 tensor_scalar ops (4x perf mode).
  - stage 2 (pred = h @ W2): [128,6] block-diag W2 stationary placed on 4
    independent PE column strips via tile_position=(0,32r) so 4 matmuls
    stream concurrently. Moving operand reads h with the (l,s) interleave
    so PSUM rows are whole output rows.
  - ACT copies pred (PSUM) -> bf16 staging; two DMA queues (sync/gpsimd)
    write 48KB batched descriptors straight to DRAM. Host upcasts + b2.
"""

import numpy as np
import ml_dtypes

import concourse.bass as bass
import concourse.bacc as bacc
import concourse.mybir as mybir
import concourse.tile as tile
from concourse.bass_utils import run_bass_kernel_spmd

# Problem constants (hardcoded per contract)
B, C, H, W = 4, 64, 128, 128
OUT = 512
NF = 64
N_CORES = 8
ROWS_PER_CORE = H // 2          # 64 input rows per core
NBLK = 2                        # blocks per core; block = 16 A-rows + 16 B-rows
BI = 16                         # input rows per group per block
REL = np.array([-0.75, -0.25, 0.25, 0.75], dtype=np.float32)

_CACHE = {}


def _build_program():
    if "nc" in _CACHE:
        return _CACHE["nc"]

    fp32 = mybir.dt.float32
    bf16 = mybir.dt.bfloat16
    nc = bacc.Bacc("TRN2", target_bir_lowering=False, debug=False,
                   num_devices=N_CORES)

    x_d = nc.dram_tensor("x", [C, ROWS_PER_CORE, W], bf16, kind="ExternalInput")
    w1_d = nc.dram_tensor("w1diag", [128, 128], bf16, kind="ExternalInput")
    w2_d = nc.dram_tensor("w2diag", [128, 6], bf16, kind="ExternalInput")
    crs_d = nc.dram_tensor("crsT", [128, 16], fp32, kind="ExternalInput")
    out_d = nc.dram_tensor("out", [3, 4 * ROWS_PER_CORE, OUT], bf16,
                           kind="ExternalOutput")

    with tile.TileContext(nc) as tc:
        with (
            tc.tile_pool(name="consts", bufs=1) as consts,
            tc.tile_pool(name="xin", bufs=2) as xin,
            tc.tile_pool(name="fbf", bufs=2) as fbfp,
            tc.tile_pool(name="hbuf", bufs=2) as hbuf,
            tc.tile_pool(name="stage", bufs=2) as stage,
            tc.tile_pool(name="fpsum", bufs=2, space=bass.MemorySpace.PSUM) as fpsum,
            tc.tile_pool(name="ppsum", bufs=2, space=bass.MemorySpace.PSUM) as ppsum,
        ):
            w1_sb = consts.tile([128, 128], bf16)
            w2_sb = consts.tile([128, 6], bf16)
            crs_sb = consts.tile([128, 16], fp32)

            x_tiles, f_tiles, fbf_tiles = [], [], []

            def load_x(b, eng_a=None, eng_b=None):
                xt = xin.tile([128, BI, W], bf16, tag="xt")
                (eng_a or nc.sync).dma_start(
                    xt[0:64, :, :], x_d[:, BI * b:BI * (b + 1), :])
                (eng_b or nc.gpsimd).dma_start(
                    xt[64:128, :, :],
                    x_d[:, 32 + BI * b:32 + BI * (b + 1), :])
                x_tiles.append(xt)

            # x(0) entirely on the sync HWDGE queue (lower completion
            # latency than SWDGE -> stage-1 starts sooner); w1 next on sync;
            # w2/crs + x(1) on gpsimd
            load_x(0, eng_a=nc.sync, eng_b=nc.sync)
            nc.sync.dma_start(w1_sb[:], w1_d[:])
            nc.gpsimd.dma_start(w2_sb[:], w2_d[:])
            nc.gpsimd.dma_start(crs_sb[:], crs_d[:])

            # PE clock-gate warmup: dummy matmuls on zeroed scratch while
            # the x DMA is in flight, so stage-1 runs at full 2.4 GHz
            scr = consts.tile([128, 512], bf16)
            nc.vector.memset(scr[:], 0.0)
            for _ in range(12):
                pw = fpsum.tile([128, 8, W], fp32, tag="ft")
                nc.tensor.matmul(pw[:, 0:4, :], scr[:, 0:128], scr[:, :],
                                 start=True, stop=True)

            def feat(b):
                # two 1024-col stage-1 matmuls -> F halves in PSUM
                xt = x_tiles[b]
                fs = []
                for half in range(2):
                    ft = fpsum.tile([128, 8, W], fp32, tag="ft")
                    for q in range(2):
                        sl = slice(4 * q, 4 * q + 4)
                        nc.tensor.matmul(ft[:, sl, :], w1_sb[:],
                                         xt[:, 8 * half:8 * half + 8, :][:, sl, :],
                                         start=True, stop=True)
                    fs.append(ft)
                f_tiles.append(fs)

            def fbf_copy(b):
                # ACT: PSUM fp32 -> SBUF bf16 (feeds DVE 4x relu)
                fb = fbfp.tile([128, 2 * 8 * W], bf16, tag="fb")
                for half in range(2):
                    nc.scalar.activation(
                        fb[:, 1024 * half:1024 * (half + 1)],
                        f_tiles[b][half][:, :, :],
                        mybir.ActivationFunctionType.Copy)
                fbf_tiles.append(fb)

            def body(b):
                fb = fbf_tiles[b]
                # DVE: 16 bias+relu variants, each [128, 2048] bf16 (4x mode)
                h = hbuf.tile([128, 16, 2048], bf16, tag="h")
                for v in range(16):
                    nc.vector.tensor_scalar(
                        h[:, v, :], fb[:, :],
                        crs_sb[:, v:v + 1], 0.0,
                        mybir.AluOpType.add, mybir.AluOpType.max)

                # stage 2, r-major: an r-quarter only needs relu variants
                # 4r..4r+3, so PE consumes h incrementally as DVE produces
                # it. Strip = i//4, pt slot k = i%4: partition-group G then
                # holds output rows 16G+4k+r -> contiguous DRAM rows.
                # st free layout: (k, r, col).
                st = stage.tile([102, 4, 4, OUT], bf16, tag="st")
                for r in range(4):
                    for kp in range(2):
                        pt = ppsum.tile([102, 2, OUT], fp32, tag="pt")
                        for kk in range(2):
                            for G in range(4):
                                i = 4 * G + 2 * kp + kk
                                rhs = h[:, 4 * r:4 * r + 4,
                                        128 * i:128 * (i + 1)]
                                nc.tensor.matmul(pt[32 * G:32 * G + 6, kk, :],
                                                 w2_sb[:], rhs,
                                                 start=True, stop=True,
                                                 tile_position=(0, 32 * G))
                        # un-interleave (s,l) -> (4l+s) via strided PSUM read
                        src = pt[:, :, :].rearrange("p j (s l) -> p j l s",
                                                    s=4)
                        dst = st[:, 2 * kp:2 * kp + 2, r, :]
                        if b == NBLK - 1 and r == 3 and kp == 0:
                            # DVE is done with relus by now; split the final
                            # two copies across both engines
                            nc.vector.tensor_copy(dst, src)
                        else:
                            nc.scalar.activation(
                                dst, src, mybir.ActivationFunctionType.Copy)

                # output DMAs: rows 128ab + 64b + 16G + (4k+r) are contiguous
                # 16-row runs; last block drains in k-halves
                ksplit = ((0, 4),)
                for k0, k1 in ksplit:
                    for idx, (G, ab) in enumerate(
                            (G, ab) for G in range(4) for ab in range(2)):
                        row0 = 128 * ab + 64 * b + 16 * G + 4 * k0
                        eng = nc.gpsimd if (G + ab) % 2 else nc.sync
                        eng.dma_start(
                            out_d[:, row0:row0 + 4 * (k1 - k0), :],
                            st[32 * G + 3 * ab:32 * G + 3 * ab + 3,
                               k0:k1, :, :])

            feat(0)
            fbf_copy(0)
            for b in range(NBLK):
                if b + 1 < NBLK:
                    load_x(b + 1, eng_a=nc.gpsimd, eng_b=nc.gpsimd)
                    feat(b + 1)
                    fbf_copy(b + 1)
                body(b)

    nc.compile()
    _CACHE["nc"] = nc
    return nc


def _prep_inputs(x, W1, b1, W2, b2):
    x = np.asarray(x, dtype=np.float32)
    W1 = np.asarray(W1, dtype=np.float32)
    b1 = np.asarray(b1, dtype=np.float32)
    W2 = np.asarray(W2, dtype=np.float32)

    w1c = W1[:NF]
    w1diag = np.zeros((128, 128), dtype=np.float32)
    w1diag[0:64, 0:64] = w1c
    w1diag[64:128, 64:128] = w1c

    w2diag = np.zeros((128, 6), dtype=np.float32)
    w2diag[0:64, 0:3] = W2
    w2diag[64:128, 3:6] = W2

    # c[r,s] = rel[r]*W1[64] + rel[s]*W1[65] + b1 -> [16, 64] -> [128, 16]
    crs = (REL[:, None, None] * W1[NF][None, None, :]
           + REL[None, :, None] * W1[NF + 1][None, None, :]
           + b1[None, None, :]).reshape(16, NF)
    crsT = np.ascontiguousarray(np.concatenate([crs.T, crs.T], axis=0))

    w1_bf = w1diag.astype(ml_dtypes.bfloat16)
    w2_bf = w2diag.astype(ml_dtypes.bfloat16)

    in_maps = []
    for c in range(N_CORES):
        b, half = c // 2, c % 2
        xs = np.ascontiguousarray(
            x[b, :, half * ROWS_PER_CORE:(half + 1) * ROWS_PER_CORE, :]
        ).astype(ml_dtypes.bfloat16)
        in_maps.append({"x": xs, "w1diag": w1_bf, "w2diag": w2_bf,
                        "crsT": crsT})
    return in_maps


def _gather(results, b2):
    full = np.empty((B, 3, OUT, OUT), dtype=np.float32)
    for c in range(N_CORES):
        b, half = c // 2, c % 2
        full[b, :, half * (OUT // 2):(half + 1) * (OUT // 2), :] = \
            np.asarray(results[c]["out"]).astype(np.float32)
    b2 = np.asarray(b2, dtype=np.float32)
    if np.any(b2):
        full += b2.reshape(1, 3, 1, 1)
    return full


def run(trace=False, **inputs):
    nc = _build_program()
    in_maps = _prep_inputs(inputs["x"], inputs["W1"], inputs["b1"],
                           inputs["W2"], inputs["b2"])
    res = run_bass_kernel_spmd(nc, in_maps, list(range(N_CORES)), trace=trace)
    return _gather(res.results, inputs["b2"]), res


def kernel(**inputs):
    out, _ = run(trace=False, **inputs)
    return out
